# revision 17
# baseline (speedup 1.0000x reference)
"""Trainium2 Bass kernel for multi-head self-attention (dense transformer block).

Reference computation (per batch element b):
    qkv  = x @ w_in.T + b_in                      # [N, 3C]
    q,k,v per head (H=16, D=64)
    S    = (q @ k.T) * D**-0.5
    P    = softmax(S)                             # rows over keys
    attn = P @ v                                  # [N, C] after head merge
    y    = attn @ w_out.T + b_out

Sharding: data-parallel over batch. B=8 maps one batch element per NeuronCore.

Per-core dataflow (bf16 matmuls, fp32 PSUM accumulation; the softmax
normalization chain runs in fp32r for accuracy):
  - host pre-transposes x[b] -> xT [C, N] and the weights (w_inT [C,3C],
    w_outT [C,C]) and casts them to bf16, so every matmul operand loads with
    a contiguous DMA at full PE rate (1 elem/cycle, FWL weight loads).
  - in_proj produces qT,kT in feature-major layout [f, n] (f on partitions,
    bias fused into the PSUM eviction as a per-partition DVE add) and v in
    token-major layout [n, f] (bias via a K=1 ones-matmul into the PSUM
    accumulation).  v is stored per head with a 65th all-ones column.
  - attention computes S^T per head ([keys, queries] layout) so that
    P^T = exp(S^T * scale) comes out with keys on partitions, ready to be the
    moving operand of the P@V matmul (contraction over keys).  Softmax skips
    the max-subtraction (scores are O(5) so exp is safe in fp32), which lets
    exp fuse into the PSUM->SBUF eviction on the scalar engine as one
    [128, 1024] op per (head, key-tile).  The P@V matmul uses the [v | ones]
    stationary (M=65): row 64 of the output is the softmax denominator,
    computed for free on the tensor engine.  Because the denominator sums the
    same bf16-rounded P used by P@V, the normalization is exact with respect
    to the rounding.
  - normalization: 1/rowsum on DVE (fp32r), broadcast across partitions via a
    K=1 fp32r ones-matmul (fp32r matmuls must write PSUM at partition 0, so
    head 1's normalized tile takes an SBUF->SBUF DMA hop to partitions
    64-127).
  - out_proj consumes the head-merged attnT [c, n] directly as the stationary
    operand; bias again via a K=1 ones-matmul.
  - the q/k projection matmuls of head pair hp+1 are interleaved into the
    attention of pair hp so the tensor engine stays saturated while the
    scalar engine drains the exp evictions (keeps the HAM clock gate at 8/8).
"""
import numpy as np
from contextlib import ExitStack

import concourse.bass as bass  # noqa: F401
from concourse import bacc
import concourse.tile as tile
from concourse import mybir

F32 = mybir.dt.float32
F32R = mybir.dt.float32r
BF16 = mybir.dt.bfloat16
EXP = mybir.ActivationFunctionType.Exp

B = 8
N = 1024          # tokens
C = 1024          # hidden
H = 16            # heads
D = C // H        # 64
F3 = 3 * C
SCALE = float(D) ** -0.5
P = 128
CT = C // P       # 8 contraction tiles over C
NT = N // P       # 8 token tiles
HP = H // 2       # 8 head pairs (two heads share a 128-partition tile)
IB = 512          # query-block (matmul moving free dim)
NIB = N // IB     # 2

_CACHE = {}
LAST_EXEC_TIME_NS = None


def _build():
    nc = bacc.Bacc("TRN2", target_bir_lowering=False, debug=False)
    xT = nc.dram_tensor("xT", [C, N], BF16, kind="ExternalInput")
    w_inT = nc.dram_tensor("w_inT", [C, F3], BF16, kind="ExternalInput")
    b_qk_pm = nc.dram_tensor("b_qk_pm", [P, 2 * CT], F32, kind="ExternalInput")
    b_v_bf = nc.dram_tensor("b_v_bf", [C], BF16, kind="ExternalInput")
    b_o_bf = nc.dram_tensor("b_o_bf", [C], BF16, kind="ExternalInput")
    w_outT = nc.dram_tensor("w_outT", [C, C], BF16, kind="ExternalInput")
    ones_bf = nc.dram_tensor("ones_bf", [512], BF16, kind="ExternalInput")
    y = nc.dram_tensor("y", [N, C], F32, kind="ExternalOutput")

    with tile.TileContext(nc) as tc:
        with ExitStack() as ctx:
            consts = ctx.enter_context(tc.tile_pool(name="consts", bufs=1))
            qkp = ctx.enter_context(tc.tile_pool(name="qk", bufs=8))
            vp = ctx.enter_context(tc.tile_pool(name="v", bufs=1))
            atp = ctx.enter_context(tc.tile_pool(name="attnT", bufs=1))
            io_ps = ctx.enter_context(tc.tile_pool(name="io_ps", bufs=2, space="PSUM"))
            wbp = ctx.enter_context(tc.tile_pool(name="wB", bufs=16))
            yp = ctx.enter_context(tc.tile_pool(name="y", bufs=3))

            # ---- constants ----
            ones_bsq = consts.tile([P, P], BF16)    # all-ones square (bf16)
            nc.sync.dma_start(ones_bsq[:], ones_bf.ap()[None, 0:P].to_broadcast([P, P]))
            b_qk = consts.tile([P, 2 * CT], F32)    # q/k bias, per-partition
            nc.sync.dma_start(b_qk[:], b_qk_pm.ap())
            b_v = consts.tile([1, C], BF16)         # v bias as a row (rhs)
            nc.sync.dma_start(b_v[:], b_v_bf.ap()[None, :])
            b_o = consts.tile([1, C], BF16)         # out bias as a row (rhs)
            nc.sync.dma_start(b_o[:], b_o_bf.ap()[None, :])

            # ---- persistent big tensors ----
            # q/k in feature-major layout, one tile per (region, head pair) so
            # attention of pair hp depends only on its own projection tiles.
            qk_t = {}                           # allocated lazily, 8 slots rotate
            v_ext = vp.tile([P, NT, H, D + 1], BF16)  # [n_in, n_tile, head, d|1]
            attnT = atp.tile([P, CT, N], BF16)      # [c_in, c_tile, n]

            # ones column of v_ext (free-dim broadcast copy from ones_bsq)
            nc.vector.tensor_copy(
                v_ext[:, :, :, D:D + 1],
                ones_bsq[:, None, None, 0:1].to_broadcast([P, NT, H, 1]))

            pctx = ctx.enter_context(ExitStack())
            xp = pctx.enter_context(tc.tile_pool(name="x", bufs=1))
            wap = pctx.enter_context(tc.tile_pool(name="wA", bufs=32))
            vctx = pctx.enter_context(ExitStack())
            wvp = vctx.enter_context(tc.tile_pool(name="wv", bufs=17))
            wv_all = {}
            for fb in range(C // IB):
                for ct in range(CT):
                    wt = wvp.tile([P, IB], BF16, tag="wv",
                                  name=f"wv_{fb}_{ct}")
                    nc.sync.dma_start(
                        wt[:], w_inT.ap()[ct * P:(ct + 1) * P,
                                          2 * C + fb * IB:2 * C + (fb + 1) * IB])
                    wv_all[(fb, ct)] = wt
            xT_sb = xp.tile([P, CT, N], BF16)
            for ct in range(CT):
                nc.sync.dma_start(xT_sb[:, ct, :],
                                  xT.ap()[ct * P:(ct + 1) * P, :])

            # ---- v projection, token-major ----
            # v[n, f'] = sum_c xT[c, n] * w_inT[c, 2C+f'] + b_in[2C+f']
            for fb in range(C // IB):
                wvs = [wv_all[(fb, ct)] for ct in range(CT)]
                hs = fb * (IB // D)              # first head in this slab
                he = (fb + 1) * (IB // D)
                for nt in range(NT):
                    ps = io_ps.tile([P, IB], F32, tag="iops")
                    for ct in range(CT):
                        nc.tensor.matmul(
                            ps[:], xT_sb[:, ct, nt * P:(nt + 1) * P],
                            wvs[ct][:], start=(ct == 0), stop=False)
                    nc.tensor.matmul(
                        ps[:], ones_bsq[0:1, :], b_v[0:1, fb * IB:(fb + 1) * IB],
                        start=False, stop=True)
                    nc.vector.tensor_copy(
                        v_ext[:, nt, hs:he, 0:D],
                        ps[:].rearrange("p (h d) -> p h d", d=D))

            # ---- q/k projection for one head pair ----
            # Emits the 16 weight-tile DMAs and 4 accumulation chains
            # (2 regions x 2 query blocks) of 8 matmuls each, as 8 chunks of
            # 4 matmuls for interleaving with attention.
            def qk_proj_chunks(hp):
                for reg in range(2):
                    qk_t[(reg, hp)] = qkp.tile([P, N], BF16, tag="qk",
                                               name=f"qk_{reg}_{hp}")
                wts = {}
                for reg in range(2):
                    for ct in range(CT):
                        wt = wap.tile([P, P], BF16, tag="w")
                        nc.sync.dma_start(
                            wt[:], w_inT.ap()[ct * P:(ct + 1) * P,
                                              reg * C + hp * P:
                                              reg * C + (hp + 1) * P])
                        wts[(reg, ct)] = wt
                chains = []
                for reg in range(2):
                    for nb in range(NIB):
                        chains.append((reg, nb))

                def chunk(i):                    # i in 0..7 -> half-chain
                    reg, nb = chains[i // 2]
                    ft = reg * CT + hp
                    ps_key = (reg, nb)
                    if i % 2 == 0:
                        qk_proj_chunks.ps[ps_key] = io_ps.tile(
                            [P, IB], F32, tag="iops", name=f"qkps_{hp}_{reg}_{nb}")
                    ps = qk_proj_chunks.ps[ps_key]
                    for ct in range(4 * (i % 2), 4 * (i % 2) + 4):
                        nc.tensor.matmul(
                            ps[:], wts[(reg, ct)][:],
                            xT_sb[:, ct, nb * IB:(nb + 1) * IB],
                            start=(ct == 0), stop=(ct == CT - 1))
                    if i % 2 == 1:
                        nc.vector.tensor_scalar_add(
                            qk_t[(reg, hp)][:, nb * IB:(nb + 1) * IB], ps[:],
                            b_qk[:, ft:ft + 1])
                return chunk
            qk_proj_chunks.ps = {}

            # ---- attention for one head pair, interleaved with the q/k
            # projection matmuls of a later pair (keeps PE busy while the
            # scalar engine drains the exp evictions) ----
            vctx.close()                        # free the v-projection weights
            ptp = pctx.enter_context(tc.tile_pool(name="pt", bufs=36))
            r2p = pctx.enter_context(tc.tile_pool(name="r2", bufs=4))
            pvcp = pctx.enter_context(tc.tile_pool(name="pvc", bufs=6))
            rbp = pctx.enter_context(tc.tile_pool(name="rb", bufs=4))
            tmpp = pctx.enter_context(tc.tile_pool(name="tmp", bufs=3))
            rdp = pctx.enter_context(tc.tile_pool(name="rd", bufs=4, space="DRAM"))
            st_ps = pctx.enter_context(tc.tile_pool(name="st_ps", bufs=2, space="PSUM"))
            pv_ps = pctx.enter_context(tc.tile_pool(name="pv_ps", bufs=2, space="PSUM"))

            def norm_chain(hp, ib, pv0, pv1):
                isl = slice(ib * IB, (ib + 1) * IB)
                # evict P@V to SBUF right away so the PSUM slots free up for
                # the next accumulation; the normalization chain then runs
                # entirely off-PSUM.
                pvc0 = pvcp.tile([D + 1, IB], F32, tag="pvc")
                pvc1 = pvcp.tile([D + 1, IB], F32, tag="pvc")
                nc.vector.tensor_copy(pvc0[:], pv0[:])
                nc.vector.tensor_copy(pvc1[:], pv1[:])
                # denominator (row 64): spread over 64 partitions via a DRAM
                # bounce, reciprocal in parallel, broadcast back via DMA.
                rbs = []
                for pvc in (pvc0, pvc1):
                    sd = rdp.tile([1, IB], F32, tag="rd")
                    nc.gpsimd.dma_start(sd[:], pvc[D:D + 1, :])
                    rsp = r2p.tile([D, IB // D], F32, tag="rsp")
                    nc.gpsimd.dma_start(
                        rsp[:], sd[:].rearrange("a (p o) -> (a p) o", p=D))
                    nc.vector.reciprocal(rsp[:], rsp[:])
                    rd = rdp.tile([1, IB], F32, tag="rd")
                    nc.gpsimd.dma_start(
                        rd[:].rearrange("a (p o) -> (a p) o", p=D), rsp[:])
                    rb_sb = rbp.tile([D, IB], F32, tag="rb")
                    nc.gpsimd.dma_start(rb_sb[:], rd[:].to_broadcast([D, IB]))
                    rbs.append(rb_sb)
                rb0_sb, rb1_sb = rbs
                # normalized head outputs -> attnT
                nc.vector.tensor_tensor(
                    attnT[0:D, hp, isl], pvc0[0:D, :], rb0_sb[:],
                    mybir.AluOpType.mult)
                tmp = tmpp.tile([D, IB], BF16, tag="tmp")
                nc.vector.tensor_tensor(
                    tmp[:], pvc1[0:D, :], rb1_sb[:],
                    mybir.AluOpType.mult)
                nc.gpsimd.dma_start(attnT[D:P, hp, isl], tmp[:])

            def attn_st(hp, filler):
                # S^T tiles + fused exp -> P^T [keys, queries]; one
                # [128, 1024] psum tile and one exp per (head, key-tile).
                pts = {}
                for jt in range(NT):
                    stt = {}
                    for h in (0, 1):
                        stt[h] = st_ps.tile([P, N], F32, tag="st",
                                            name=f"st_{hp}_{jt}_{h}")
                    for ib in range(NIB):
                        for h in (0, 1):   # adjacent row-groups run packed
                            hsl = slice(h * D, (h + 1) * D)
                            nc.tensor.matmul(
                                stt[h][:, ib * IB:(ib + 1) * IB],
                                qk_t[(1, hp)][hsl, jt * P:(jt + 1) * P],
                                qk_t[(0, hp)][hsl, ib * IB:(ib + 1) * IB],
                                start=True, stop=True)
                    for h in (0, 1):
                        pt_t = ptp.tile([P, N], BF16, tag="pt")
                        nc.scalar.activation(pt_t[:], stt[h][:], EXP, scale=SCALE)
                        pts[(h, jt)] = pt_t
                    if filler is not None and jt % 2 == 1:
                        filler()
                return pts

            def attn_pv(hp, pts, filler):
                # P@V with [v | ones] stationary: row 64 = denominator
                for ib in range(NIB):
                    pv0 = pv_ps.tile([D + 1, IB], F32, tag="pv",
                                     name=f"pv0_{hp}_{ib}")
                    pv1 = pv_ps.tile([D + 1, IB], F32, tag="pv",
                                     name=f"pv1_{hp}_{ib}")
                    isl = slice(ib * IB, (ib + 1) * IB)
                    for jt in range(NT):
                        fl = dict(start=(jt == 0), stop=(jt == NT - 1))
                        nc.tensor.matmul(
                            pv0[:], v_ext[:, jt, 2 * hp, :],
                            pts[(0, jt)][:, isl], **fl)
                        nc.tensor.matmul(
                            pv1[:], v_ext[:, jt, 2 * hp + 1, :],
                            pts[(1, jt)][:, isl], **fl)
                        if filler is not None and jt % 4 == 1:
                            filler()
                    norm_chain(hp, ib, pv0, pv1)

            # software pipeline, two levels deep:
            #  - the q/k projection of pair hp+1 is interleaved into round hp
            #  - P@V of pair hp-1 runs during the score/exp stage of pair hp
            chunk0 = qk_proj_chunks(0)
            for i in range(8):
                chunk0(i)
            pending = qk_proj_chunks(1)
            pts_prev = None

            for r in range(HP + 1):
                hp_st = r if r < HP else None
                nxt = qk_proj_chunks(r + 2) if r + 2 < HP else None
                chunks = pending
                ci = [0]

                if chunks is not None:
                    def filler(chunks=chunks, ci=ci):
                        if ci[0] < 8:
                            chunks(ci[0])
                            ci[0] += 1
                else:
                    def filler():
                        # keep the PE activity monitor fed so the clock gate
                        # stays at 8/8 through the drain rounds
                        for _ in range(4):
                            nc.tensor.ldweights(ones_bsq[:, :])

                pts_cur = attn_st(hp_st, filler) if hp_st is not None else None
                if pts_prev is not None:
                    attn_pv(r - 1, pts_prev, filler)
                # flush any chunks the interleave did not reach
                if chunks is not None:
                    while ci[0] < 8:
                        chunks(ci[0])
                        ci[0] += 1
                pending = nxt
                pts_prev = pts_cur

            # prefetch all out-projection weights while attention drains
            wo_all = {}
            for cb in range(C // IB):
                for ct in range(CT):
                    wt = wbp.tile([P, IB], BF16, tag="wo",
                                  name=f"wo_{cb}_{ct}")
                    nc.sync.dma_start(
                        wt[:], w_outT.ap()[ct * P:(ct + 1) * P,
                                           cb * IB:(cb + 1) * IB])
                    wo_all[(cb, ct)] = wt
            pctx.close()                        # free x, wA, pt, ... pools
            # ---- out projection ----
            # y[n, c'] = sum_c attnT[c, n] * w_outT[c, c'] + b_out[c']
            op_ps = ctx.enter_context(tc.tile_pool(name="op_ps", bufs=5, space="PSUM"))
            for cb in range(C // IB):
                wos = [wo_all[(cb, ct)] for ct in range(CT)]
                for nt in range(NT):
                    ps = op_ps.tile([P, IB], F32, tag="ops")
                    for ct in range(CT):
                        nc.tensor.matmul(
                            ps[:], attnT[:, ct, nt * P:(nt + 1) * P],
                            wos[ct][:], start=(ct == 0), stop=False)
                    nc.tensor.matmul(
                        ps[:], ones_bsq[0:1, :], b_o[0:1, cb * IB:(cb + 1) * IB],
                        start=False, stop=True)
                    yt = yp.tile([P, IB], F32, tag="y")
                    nc.vector.tensor_copy(yt[:], ps[:])
                    nc.sync.dma_start(
                        y.ap()[nt * P:(nt + 1) * P, cb * IB:(cb + 1) * IB],
                        yt[:])

    nc.compile()
    return nc


def _get_nc():
    if "nc" not in _CACHE:
        _CACHE["nc"] = _build()
    return _CACHE["nc"]


def kernel(x, w_in, b_in, w_out, b_out):
    global LAST_EXEC_TIME_NS
    import ml_dtypes
    from concourse.bass_utils import run_bass_kernel_spmd

    bf16 = ml_dtypes.bfloat16
    x = np.asarray(x, dtype=np.float32)
    w_in = np.asarray(w_in, dtype=np.float32)
    b_in = np.asarray(b_in, dtype=np.float32)
    w_out = np.asarray(w_out, dtype=np.float32)
    b_out = np.asarray(b_out, dtype=np.float32)

    w_inT = np.ascontiguousarray(w_in.T).astype(bf16)
    w_outT = np.ascontiguousarray(w_out.T).astype(bf16)
    ones_b = np.ones(512, dtype=bf16)
    ones_f = np.ones(512, dtype=np.float32)
    b_qk_pm = np.ascontiguousarray(b_in[0:2 * C].reshape(2 * CT, P).T)
    b_v = b_in[2 * C:F3].astype(bf16)
    b_o = b_out.astype(bf16)

    in_maps = []
    for b in range(B):
        in_maps.append({
            "xT": np.ascontiguousarray(x[b].T).astype(bf16),
            "w_inT": w_inT,
            "b_qk_pm": b_qk_pm,
            "b_v_bf": b_v,
            "b_o_bf": b_o,
            "w_outT": w_outT,
            "ones_bf": ones_b,
            "ones_fr": ones_f,
        })

    nc = _get_nc()

    trace = False
    import os
    if os.environ.get("BASS_KERNEL_TRACE") == "1":
        try:
            import profshim  # noqa: F401
            trace = True
        except Exception:
            trace = False

    res = run_bass_kernel_spmd(nc, in_maps, core_ids=list(range(B)),
                               trace=trace)
    LAST_EXEC_TIME_NS = res.exec_time_ns
    out = np.stack([res.results[b]["y"] for b in range(B)], axis=0)
    return out
